# revision 36
# baseline (speedup 1.0000x reference)
"""Trainium2 Bass kernel for the 3-layer MLP encode/decode forward pass.

Computation (B = 65536):
    d_i = pinv(W_i)                       (host, negligible)
    h = lrelu(x @ W1.T)                   [B, 128]
    h = lrelu(h @ W2.T)                   [B, 64]
    h = h @ W3.T                          [B, 16]
    h = lrelu(h @ d3.T)                   [B, 64]   (folded: lrelu((d3@W3) @ h2))
    h = lrelu(h @ d2.T)                   [B, 128]
    out = h @ d1.T                        [B, 784]

Sharding: pure data-parallel — 8 cores x 8192 batch rows each; the tiny
weights (and host-side pinv) are replicated.

Per-core layout: activations are kept feature-major ([feat, batch]) so
TensorE contracts over features.  x is transposed on-chip via PE
transpose-mode (exact fp32).  The final layer swaps operand roles
(stationary = activation tile, moving = d1.T) so the output lands
batch-major in PSUM — no output transpose.  Matmuls run as float32r
(full PE rate at moving-N >= 256, ~tf32 rounding).

DMA: one 1.6MB transfer per 512-row tile each way ([128, 4*784] with 4
batch rows per partition — batch order inside a tile is permuted, which
cancels between the input transposes and the output writeback).
"""

import numpy as np

B = 65536
N_CORES = 8
B_LOC = B // N_CORES  # 8192
D0, D1, D2, D3 = 784, 128, 64, 16
KCH = 112          # 784 = 7 * 112 contraction chunks for layer 1
NKC = D0 // KCH    # 7
TILE = 512         # moving free dim per matmul (one fp32 PSUM bank)
SUB = 128          # batch sub-tile (partition dim of x / out tiles)
NSUB = TILE // SUB  # 4
HALF = D0 // 2     # 392


def _build_nc(b_loc=B_LOC, *args, **kwargs):
    if kwargs.pop("arch", "v2") == "v3":
        return _build_v3(b_loc, **kwargs)
    return _build_v2(b_loc, *args, **kwargs)


def _build_v2(b_loc=B_LOC, mm_dt_name="float32r", last_dt_name="float32r",
              act_name="Lrelu", repeat=1, r_xpose=False, split_ocopy=False,
              bf16_out=False, xt_on_act=False, xin_bufs=4, outp_bufs=4,
              xtp_bufs=14, acts_bufs=2, out_dma_eng="sync", in_dma_eng="sync",
              alloc_mode="stack", staggered=False, no_l5=False,
              split_in=False, ps_rebal=False, hi_in=False, n_devices=N_CORES,
              out_dt_name=None, ocopy_dve=(), mode="full", dma_tiles=1):
    import contextlib
    import concourse.tile as tile
    from concourse import bacc, mybir

    mm_dt = getattr(mybir.dt, mm_dt_name)
    last_dt = getattr(mybir.dt, last_dt_name)
    f32 = mybir.dt.float32
    LRELU = getattr(mybir.ActivationFunctionType, act_name)
    COPY = mybir.ActivationFunctionType.Copy

    nc = bacc.Bacc(trn_type="TRN2", target_bir_lowering=False, debug=False,
                   num_devices=n_devices)

    x = nc.declare_dram_parameter("x", [b_loc, D0], f32, isOutput=False).ap()
    w1t = nc.declare_dram_parameter("w1t", [D0, D1], mm_dt, isOutput=False).ap()
    w2t = nc.declare_dram_parameter("w2t", [D1, D2], mm_dt, isOutput=False).ap()
    m3t = nc.declare_dram_parameter("m3t", [D2, D2], mm_dt, isOutput=False).ap()
    d2t = nc.declare_dram_parameter("d2t", [D2, D1], mm_dt, isOutput=False).ap()
    d1t = nc.declare_dram_parameter("d1t", [D1, D0], last_dt, isOutput=False).ap()
    ident = nc.declare_dram_parameter("ident", [SUB, SUB], f32, isOutput=False).ap()
    if out_dt_name is not None:
        out_dt = getattr(mybir.dt, out_dt_name)
    else:
        out_dt = mybir.dt.bfloat16 if bf16_out else f32
    out = nc.declare_dram_parameter("out", [b_loc, D0], out_dt, isOutput=True).ap()

    n_tiles = b_loc // TILE
    # row = tile*512 + p*4 + s  (4 rows per partition -> one 1.6MB DMA per tile)
    x_r = x.rearrange("(n p s) f -> n p (s f)", p=SUB, s=NSUB)
    out_r = out.rearrange("(n p s) f -> n p (s f)", p=SUB, s=NSUB)

    with tile.TileContext(nc, num_cores=n_devices, pool_alloc_mode=alloc_mode) as tc:
        with (
            tc.tile_pool(name="consts", bufs=1) as consts,
            tc.tile_pool(name="xin", bufs=xin_bufs) as xin,
            tc.tile_pool(name="xtp", bufs=xtp_bufs) as xtp,
            tc.tile_pool(name="acts", bufs=acts_bufs) as acts,
            tc.tile_pool(name="outp", bufs=outp_bufs) as outp,
            tc.tile_pool(name="psT", bufs=3 if ps_rebal else 2,
                         space="PSUM") as psT,
            tc.tile_pool(name="psMM", bufs=3 if ps_rebal else 2,
                         space="PSUM") as psMM,
            tc.tile_pool(name="psO", bufs=1 if ps_rebal else 2,
                         space="PSUM") as psO,
        ):
            # --- constants ---
            w1t_sb = consts.tile([KCH, NKC, D1], mm_dt)
            nc.sync.dma_start(out=w1t_sb, in_=w1t.rearrange("(c p) m -> p c m", p=KCH))
            w2t_sb = consts.tile([D1, D2], mm_dt)
            nc.sync.dma_start(out=w2t_sb, in_=w2t)
            m3t_sb = consts.tile([D2, D2], mm_dt)
            nc.sync.dma_start(out=m3t_sb, in_=m3t)
            d2t_sb = consts.tile([D2, D1], mm_dt)
            nc.sync.dma_start(out=d2t_sb, in_=d2t)
            d1t_sb = consts.tile([D1, D0], last_dt)
            nc.sync.dma_start(out=d1t_sb, in_=d1t)
            id_sb = consts.tile([SUB, SUB], f32)
            nc.sync.dma_start(out=id_sb, in_=ident)
            id_r = id_sb.bitcast(mybir.dt.float32r)

            do_in = mode in ("full", "dma", "dma_in", "dma_in_cast")
            do_out = mode in ("full", "dma", "dma_out")
            do_compute = mode in ("full", "compute", "xpose", "mm_act")
            o_src = x_c = xt_c = None
            x_rb = x.rearrange("(n p s) f -> n p (s f)", p=SUB,
                               s=NSUB * dma_tiles)
            out_rb = out.rearrange("(n p s) f -> n p (s f)", p=SUB,
                                   s=NSUB * dma_tiles)
            if do_out and not do_compute:
                o_src = consts.tile([SUB, NSUB * dma_tiles * D0], out_dt)
                nc.vector.memset(o_src, 0.25)
            if do_compute and not do_in and mode != "mm_act":
                x_c = consts.tile([SUB, NSUB, D0], f32)
                nc.vector.memset(x_c, 0.125)
            if mode == "mm_act":
                xt_c = [consts.tile([KCH, TILE], mm_dt, name=f"xt_c{i}")
                        for i in range(NKC)]
                for xtc in xt_c:
                    nc.vector.memset(xtc, 0.1)

            rep_ctx = (tc.For_i(0, repeat, 1, staggered_reset=staggered)
                       if repeat > 1 else contextlib.nullcontext())
            with rep_ctx:
              for t in range(n_tiles):
                if not do_compute:
                    if t % dma_tiles:
                        continue
                    tb = t // dma_tiles
                    if mode == "dma_in_cast":
                        x_sb = xin.tile([SUB, NSUB * dma_tiles, D0],
                                        mybir.dt.float16, tag="x", name="x_sb16")
                        nc.gpsimd.dma_start(out=x_sb, in_=x_rb[tb])
                    elif do_in:
                        x_sb = xin.tile([SUB, NSUB * dma_tiles, D0], f32,
                                        tag="x", name="x_sbb")
                        getattr(nc, in_dma_eng if in_dma_eng != "alt" else "sync"
                                ).dma_start(out=x_sb, in_=x_rb[tb])
                    if do_out:
                        getattr(nc, out_dma_eng).dma_start(out=out_rb[tb],
                                                           in_=o_src)
                    continue
                # --- load 512 rows in one DMA: [128, 4, 784] ---
                if do_in:
                    x_sb = xin.tile([SUB, NSUB, D0], f32, tag="x", name="x_sb")
                else:
                    x_sb = x_c
                if not do_in:
                    pass
                elif hi_in:
                    with tc.high_priority():
                        nc.sync.dma_start(out=x_sb, in_=x_r[t])
                elif split_in:
                    xr3 = x_r[t].rearrange("p (s f) -> p s f", s=NSUB)
                    nc.sync.dma_start(out=x_sb[:, 0:2, :], in_=xr3[:, 0:2, :])
                    nc.sync.dma_start(out=x_sb[:, 2:4, :], in_=xr3[:, 2:4, :])
                elif in_dma_eng == "alt":
                    (nc.sync if t % 2 == 0 else nc.scalar).dma_start(
                        out=x_sb, in_=x_r[t])
                else:
                    getattr(nc, in_dma_eng).dma_start(out=x_sb, in_=x_r[t])

                # --- PE-transpose to feature-major: 7 chunks of [112, 512] ---
                xt_sb = []
                for c in range(NKC) if mode != "mm_act" else []:
                    tp = psT.tile([KCH, TILE], f32, tag="psT")
                    for s in range(NSUB):
                        if r_xpose:
                            nc.tensor.transpose(
                                out=tp[:, s * SUB:(s + 1) * SUB]
                                    .bitcast(mybir.dt.float32r),
                                in_=x_sb[:, s, c * KCH:(c + 1) * KCH]
                                    .bitcast(mybir.dt.float32r),
                                identity=id_r,
                            )
                        else:
                            nc.tensor.transpose(
                                out=tp[:, s * SUB:(s + 1) * SUB],
                                in_=x_sb[:, s, c * KCH:(c + 1) * KCH],
                                identity=id_sb,
                            )
                    xt = xtp.tile([KCH, TILE], mm_dt, tag="xt")
                    on_act = (c in xt_on_act if isinstance(xt_on_act, (tuple, list))
                              else xt_on_act)
                    if on_act:
                        nc.scalar.activation(out=xt, in_=tp, func=COPY)
                    else:
                        nc.vector.tensor_copy(xt, tp)
                    xt_sb.append(xt)
                if mode == "xpose":
                    continue
                if mode == "mm_act":
                    xt_sb = xt_c

                # --- L1: h1 = lrelu(W1 @ xT)  [128, 512] ---
                h1_ps = psMM.tile([D1, TILE], f32, tag="mm")
                for c in range(NKC):
                    nc.tensor.matmul(h1_ps, lhsT=w1t_sb[:, c, :], rhs=xt_sb[c],
                                     start=(c == 0), stop=(c == NKC - 1))
                h1_sb = acts.tile([D1, TILE], mm_dt, tag="h1")
                nc.scalar.activation(out=h1_sb, in_=h1_ps, func=LRELU, alpha=0.01)

                # --- L2: h2 = lrelu(W2 @ h1)  [64, 512] ---
                h2_ps = psMM.tile([D2, TILE], f32, tag="mm")
                nc.tensor.matmul(h2_ps, lhsT=w2t_sb, rhs=h1_sb,
                                 start=True, stop=True)
                h2_sb = acts.tile([D2, TILE], mm_dt, tag="h2")
                nc.scalar.activation(out=h2_sb, in_=h2_ps, func=LRELU, alpha=0.01)

                # --- L3 folded: g3 = lrelu((d3 @ W3) @ h2)  [64, 512] ---
                g3_ps = psMM.tile([D2, TILE], f32, tag="mm")
                nc.tensor.matmul(g3_ps, lhsT=m3t_sb, rhs=h2_sb,
                                 start=True, stop=True)
                g3_sb = acts.tile([D2, TILE], mm_dt, tag="g3")
                nc.scalar.activation(out=g3_sb, in_=g3_ps, func=LRELU, alpha=0.01)

                # --- L4: g2 = lrelu(d2 @ g3)  [128, 512] ---
                g2_ps = psMM.tile([D1, TILE], f32, tag="mm")
                nc.tensor.matmul(g2_ps, lhsT=d2t_sb, rhs=g3_sb,
                                 start=True, stop=True)
                g2_sb = acts.tile([D1, TILE], last_dt, tag="g2")
                nc.scalar.activation(out=g2_sb, in_=g2_ps, func=LRELU, alpha=0.01)

                # --- L5: out = g2.T @ d1.T, batch-major via stationary swap.
                # Two matmuls into one 2-bank PSUM tile ([:, :392] in bank 0,
                # [:, 512:904] in bank 1), one strided ACT copy out. ---
                o_sb = outp.tile([SUB, NSUB, D0], out_dt, tag="o")
                if no_l5:
                    nc.vector.tensor_copy(o_sb, x_sb)
                for s in range(NSUB) if not no_l5 else []:
                    g2c = g2_sb[:, s * SUB:(s + 1) * SUB]
                    po = psO.tile([SUB, 1024], f32, tag="po")
                    nc.tensor.matmul(po[:, :HALF], lhsT=g2c, rhs=d1t_sb[:, :HALF],
                                     start=True, stop=True)
                    nc.tensor.matmul(po[:, 512:512 + HALF], lhsT=g2c,
                                     rhs=d1t_sb[:, HALF:], start=True, stop=True)
                    po_v = po.rearrange("p (b r) -> p b r", b=2)[:, :, :HALF]
                    o_v = o_sb[:, s, :].rearrange("p (b r) -> p b r", b=2)
                    if (split_ocopy and s % 2 == 1) or s in ocopy_dve:
                        nc.vector.tensor_copy(o_v, po_v)
                    else:
                        nc.scalar.activation(out=o_v, in_=po_v, func=COPY)
                if do_out:
                    getattr(nc, out_dma_eng).dma_start(out=out_r[t], in_=o_sb)

    nc.finalize()
    return nc


def _build_v3(b_loc=B_LOC, repeat=1, n_devices=N_CORES, mm_dt_name="float32r",
              out_dt_name="float16", out_dma_eng="gpsimd", in_bufs=2,
              outp_bufs=2, xtp_bufs=14, acts_bufs=2, ocopy_dve=(),
              xt_act=(5, 6), staggered=False, interleave=True, sup_tiles=4,
              mode="full"):
    """Software-pipelined variant: 1024-row super-tile DMAs (amortize the
    ~2.7us per-DMA fixed cost), input on the sync HWDGE ring, fp16 output on
    the scalar HWDGE ring, and the next tile's PE transposes interleaved
    between the current tile's layer matmuls so the PE stream stays dense
    (avoids HAM down-throttle to 1.2GHz)."""
    import contextlib
    import concourse.tile as tile
    from concourse import bacc, mybir

    mm_dt = getattr(mybir.dt, mm_dt_name)
    f32 = mybir.dt.float32
    out_dt = getattr(mybir.dt, out_dt_name)
    LRELU = mybir.ActivationFunctionType.Lrelu
    COPY = mybir.ActivationFunctionType.Copy

    SUP = sup_tiles * NSUB    # batch rows per partition per super-tile
    n_sup = b_loc // (SUB * SUP)
    n_tiles = sup_tiles * n_sup

    nc = bacc.Bacc(trn_type="TRN2", target_bir_lowering=False, debug=False,
                   num_devices=n_devices)

    x = nc.declare_dram_parameter("x", [b_loc, D0], f32, isOutput=False).ap()
    w1t = nc.declare_dram_parameter("w1t", [D0, D1], mm_dt, isOutput=False).ap()
    w2t = nc.declare_dram_parameter("w2t", [D1, D2], mm_dt, isOutput=False).ap()
    m3t = nc.declare_dram_parameter("m3t", [D2, D2], mm_dt, isOutput=False).ap()
    d2t = nc.declare_dram_parameter("d2t", [D2, D1], mm_dt, isOutput=False).ap()
    d1t = nc.declare_dram_parameter("d1t", [D1, D0], mm_dt, isOutput=False).ap()
    ident = nc.declare_dram_parameter("ident", [SUB, SUB], f32, isOutput=False).ap()
    out = nc.declare_dram_parameter("out", [b_loc, D0], out_dt, isOutput=True).ap()

    # row = k*1024 + p*8 + s; the in-tile batch permutation cancels between
    # input transposes and output writeback (same (p, s) map both sides).
    x_r = x.rearrange("(n p s) f -> n p (s f)", p=SUB, s=SUP)
    out_r = out.rearrange("(n p s) f -> n p (s f)", p=SUB, s=SUP)

    with tile.TileContext(nc, num_cores=n_devices) as tc:
        with (
            tc.tile_pool(name="consts", bufs=1) as consts,
            tc.tile_pool(name="xin", bufs=in_bufs) as xin,
            tc.tile_pool(name="xtp", bufs=xtp_bufs) as xtp,
            tc.tile_pool(name="acts", bufs=acts_bufs) as acts,
            tc.tile_pool(name="outp", bufs=outp_bufs) as outp,
            tc.tile_pool(name="psT", bufs=2, space="PSUM") as psT,
            tc.tile_pool(name="psMM", bufs=2, space="PSUM") as psMM,
            tc.tile_pool(name="psO", bufs=2, space="PSUM") as psO,
        ):
            w1t_sb = consts.tile([KCH, NKC, D1], mm_dt)
            nc.sync.dma_start(out=w1t_sb, in_=w1t.rearrange("(c p) m -> p c m", p=KCH))
            w2t_sb = consts.tile([D1, D2], mm_dt)
            nc.sync.dma_start(out=w2t_sb, in_=w2t)
            m3t_sb = consts.tile([D2, D2], mm_dt)
            nc.sync.dma_start(out=m3t_sb, in_=m3t)
            d2t_sb = consts.tile([D2, D1], mm_dt)
            nc.sync.dma_start(out=d2t_sb, in_=d2t)
            d1t_sb = consts.tile([D1, D0], mm_dt)
            nc.sync.dma_start(out=d1t_sb, in_=d1t)
            id_sb = consts.tile([SUB, SUB], f32)
            nc.sync.dma_start(out=id_sb, in_=ident)

            rep_ctx = (tc.For_i(0, repeat, 1, staggered_reset=staggered)
                       if repeat > 1 else contextlib.nullcontext())
            with rep_ctx:
                x_sb = {}      # super index -> SBUF tile
                xt_of = {}     # tile index -> list of 7 xt chunks

                def in_dma(k):
                    xk = xin.tile([SUB, SUP, D0], f32, tag="x")
                    nc.sync.dma_start(out=xk, in_=x_r[k])
                    x_sb[k] = xk

                def xp_chunk(g, c):
                    """Transpose chunk c of compute tile g and copy to SBUF."""
                    base = NSUB * (g % sup_tiles)
                    xg = x_sb[g // sup_tiles]
                    tp = psT.tile([KCH, TILE], f32, tag="psT")
                    for s in range(NSUB):
                        nc.tensor.transpose(
                            out=tp[:, s * SUB:(s + 1) * SUB],
                            in_=xg[:, base + s, c * KCH:(c + 1) * KCH],
                            identity=id_sb,
                        )
                    xt = xtp.tile([KCH, TILE], mm_dt, tag="xt")
                    if c in xt_act:
                        nc.scalar.activation(out=xt, in_=tp, func=COPY)
                    else:
                        nc.vector.tensor_copy(xt, tp)
                    xt_of.setdefault(g, []).append(xt)

                in_dma(0)
                for c in range(NKC):
                    xp_chunk(0, c)

                for g in range(n_tiles):
                    nxt = g + 1 if g + 1 < n_tiles else None
                    if g % sup_tiles == 0:
                        o_sb = outp.tile([SUB, sup_tiles, NSUB, D0], out_dt,
                                         tag="o")
                        if g // sup_tiles + 1 < n_sup:
                            in_dma(g // sup_tiles + 1)

                    xt_cur = xt_of.pop(g)

                    # --- L1: h1 = lrelu(W1 @ xT)  [128, 512] ---
                    h1_ps = psMM.tile([D1, TILE], f32, tag="mm")
                    for c in range(NKC):
                        nc.tensor.matmul(h1_ps, lhsT=w1t_sb[:, c, :],
                                         rhs=xt_cur[c],
                                         start=(c == 0), stop=(c == NKC - 1))
                    h1_sb = acts.tile([D1, TILE], mm_dt, tag="h1")
                    nc.scalar.activation(out=h1_sb, in_=h1_ps, func=LRELU,
                                         alpha=0.01)
                    if interleave and nxt is not None:
                        xp_chunk(nxt, 0)
                        xp_chunk(nxt, 1)

                    # --- L2 ---
                    h2_ps = psMM.tile([D2, TILE], f32, tag="mm")
                    nc.tensor.matmul(h2_ps, lhsT=w2t_sb, rhs=h1_sb,
                                     start=True, stop=True)
                    h2_sb = acts.tile([D2, TILE], mm_dt, tag="h2")
                    nc.scalar.activation(out=h2_sb, in_=h2_ps, func=LRELU,
                                         alpha=0.01)
                    if interleave and nxt is not None:
                        xp_chunk(nxt, 2)

                    # --- L3 folded ---
                    g3_ps = psMM.tile([D2, TILE], f32, tag="mm")
                    nc.tensor.matmul(g3_ps, lhsT=m3t_sb, rhs=h2_sb,
                                     start=True, stop=True)
                    g3_sb = acts.tile([D2, TILE], mm_dt, tag="g3")
                    nc.scalar.activation(out=g3_sb, in_=g3_ps, func=LRELU,
                                         alpha=0.01)
                    if interleave and nxt is not None:
                        xp_chunk(nxt, 3)

                    # --- L4 ---
                    g2_ps = psMM.tile([D1, TILE], f32, tag="mm")
                    nc.tensor.matmul(g2_ps, lhsT=d2t_sb, rhs=g3_sb,
                                     start=True, stop=True)
                    g2_sb = acts.tile([D1, TILE], mm_dt, tag="g2")
                    nc.scalar.activation(out=g2_sb, in_=g2_ps, func=LRELU,
                                         alpha=0.01)
                    if interleave and nxt is not None:
                        xp_chunk(nxt, 4)

                    # --- L5: batch-major via stationary swap ---
                    for s in range(NSUB):
                        g2c = g2_sb[:, s * SUB:(s + 1) * SUB]
                        po = psO.tile([SUB, 1024], f32, tag="po")
                        nc.tensor.matmul(po[:, :HALF], lhsT=g2c,
                                         rhs=d1t_sb[:, :HALF],
                                         start=True, stop=True)
                        nc.tensor.matmul(po[:, 512:512 + HALF], lhsT=g2c,
                                         rhs=d1t_sb[:, HALF:],
                                         start=True, stop=True)
                        po_v = po.rearrange("p (b r) -> p b r", b=2)[:, :, :HALF]
                        o_v = o_sb[:, g % sup_tiles, s, :].rearrange(
                            "p (b r) -> p b r", b=2)
                        if s in ocopy_dve:
                            nc.vector.tensor_copy(o_v, po_v)
                        else:
                            nc.scalar.activation(out=o_v, in_=po_v, func=COPY)
                        if interleave and nxt is not None and s < 2:
                            xp_chunk(nxt, 5 + s)
                    if not interleave and nxt is not None:
                        for c in range(NKC):
                            xp_chunk(nxt, c)

                    if g % sup_tiles == sup_tiles - 1:
                        getattr(nc, out_dma_eng).dma_start(
                            out=out_r[g // sup_tiles], in_=o_sb)

    nc.finalize()
    return nc


def _host_weights(W1, W2, W3):
    def pinv(W):
        u, s, vh = np.linalg.svd(W.astype(np.float64), full_matrices=False)
        return (vh.T * (1.0 / s)) @ u.T

    d1, d2, d3 = pinv(W1), pinv(W2), pinv(W3)
    f = np.float32
    return {
        "w1t": np.ascontiguousarray(W1.T, dtype=f),
        "w2t": np.ascontiguousarray(W2.T, dtype=f),
        "m3t": np.ascontiguousarray((d3 @ W3.astype(np.float64)).T, dtype=f),
        "d2t": np.ascontiguousarray(d2.T, dtype=f),
        "d1t": np.ascontiguousarray(d1.T, dtype=f),
        "ident": np.eye(SUB, dtype=f),
    }


_NC_CACHE = {}

# Tuned configuration: fp16 output (tolerance is 2e-2; fp16 adds ~3e-4),
# output DMA on the otherwise-idle GPSIMD queue, and copy work rebalanced
# off the DVE (whose ops pay a pipeline-DRAIN ≈ op duration): DVE keeps 5
# transpose-chunk copies; ACT takes 2 chunk copies + all 4 output copies.
CONFIG = dict(
    arch="v2",
    out_dt_name="float16",
    out_dma_eng="gpsimd",
    ocopy_dve=(),
    xt_on_act=(5, 6),
)


def _get_nc(key=None):
    if key not in _NC_CACHE:
        _NC_CACHE[key] = _build_nc(B_LOC, **CONFIG)
    return _NC_CACHE[key]


def kernel(x, W1, W2, W3):
    from concourse.bass_utils import run_bass_kernel_spmd

    x = np.ascontiguousarray(x, dtype=np.float32)
    w = _host_weights(np.asarray(W1), np.asarray(W2), np.asarray(W3))
    nc = _get_nc()
    in_maps = [
        {"x": x[i * B_LOC:(i + 1) * B_LOC], **w} for i in range(N_CORES)
    ]
    res = run_bass_kernel_spmd(nc, in_maps, core_ids=list(range(N_CORES)))
    return np.concatenate(
        [np.asarray(res.results[i]["out"], dtype=np.float32) for i in range(N_CORES)],
        axis=0,
    )



# revision 39
# speedup vs baseline: 1.1435x; 1.1435x over previous
"""Trainium2 Bass kernel for the 3-layer MLP encode/decode forward pass.

Computation (B = 65536):
    d_i = pinv(W_i)                       (host, negligible)
    h = lrelu(x @ W1.T)                   [B, 128]
    h = lrelu(h @ W2.T)                   [B, 64]
    h = h @ W3.T                          [B, 16]
    h = lrelu(h @ d3.T)                   [B, 64]   (folded: lrelu((d3@W3) @ h2))
    h = lrelu(h @ d2.T)                   [B, 128]
    out = h @ d1.T                        [B, 784]

Sharding: pure data-parallel — 8 cores x 8192 batch rows each; the tiny
weights (and host-side pinv) are replicated.

Per-core layout: activations are kept feature-major ([feat, batch]) so
TensorE contracts over features.  x is transposed on-chip via PE
transpose-mode (exact fp32).  The final layer swaps operand roles
(stationary = activation tile, moving = d1.T) so the output lands
batch-major in PSUM — no output transpose.  Matmuls run as float32r
(full PE rate at moving-N >= 256, ~tf32 rounding).

DMA: one 1.6MB transfer per 512-row tile each way ([128, 4*784] with 4
batch rows per partition — batch order inside a tile is permuted, which
cancels between the input transposes and the output writeback).
"""

import numpy as np

B = 65536
N_CORES = 8
B_LOC = B // N_CORES  # 8192
D0, D1, D2, D3 = 784, 128, 64, 16
KCH = 112          # 784 = 7 * 112 contraction chunks for layer 1
NKC = D0 // KCH    # 7
TILE = 512         # moving free dim per matmul (one fp32 PSUM bank)
SUB = 128          # batch sub-tile (partition dim of x / out tiles)
NSUB = TILE // SUB  # 4
HALF = D0 // 2     # 392


def _build_nc(b_loc=B_LOC, *args, **kwargs):
    arch = kwargs.pop("arch", "v2")
    if arch == "v3":
        return _build_v3(b_loc, **kwargs)
    if arch == "v4":
        return _build_v4(b_loc, **kwargs)
    return _build_v2(b_loc, *args, **kwargs)


def _build_v2(b_loc=B_LOC, mm_dt_name="float32r", last_dt_name="float32r",
              act_name="Lrelu", repeat=1, r_xpose=False, split_ocopy=False,
              bf16_out=False, xt_on_act=False, xin_bufs=4, outp_bufs=4,
              xtp_bufs=14, acts_bufs=2, out_dma_eng="sync", in_dma_eng="sync",
              alloc_mode="stack", staggered=False, no_l5=False,
              split_in=False, ps_rebal=False, hi_in=False, n_devices=N_CORES,
              out_dt_name=None, ocopy_dve=(), mode="full", dma_tiles=1):
    import contextlib
    import concourse.tile as tile
    from concourse import bacc, mybir

    mm_dt = getattr(mybir.dt, mm_dt_name)
    last_dt = getattr(mybir.dt, last_dt_name)
    f32 = mybir.dt.float32
    LRELU = getattr(mybir.ActivationFunctionType, act_name)
    COPY = mybir.ActivationFunctionType.Copy

    nc = bacc.Bacc(trn_type="TRN2", target_bir_lowering=False, debug=False,
                   num_devices=n_devices)

    x = nc.declare_dram_parameter("x", [b_loc, D0], f32, isOutput=False).ap()
    w1t = nc.declare_dram_parameter("w1t", [D0, D1], mm_dt, isOutput=False).ap()
    w2t = nc.declare_dram_parameter("w2t", [D1, D2], mm_dt, isOutput=False).ap()
    m3t = nc.declare_dram_parameter("m3t", [D2, D2], mm_dt, isOutput=False).ap()
    d2t = nc.declare_dram_parameter("d2t", [D2, D1], mm_dt, isOutput=False).ap()
    d1t = nc.declare_dram_parameter("d1t", [D1, D0], last_dt, isOutput=False).ap()
    ident = nc.declare_dram_parameter("ident", [SUB, SUB], f32, isOutput=False).ap()
    if out_dt_name is not None:
        out_dt = getattr(mybir.dt, out_dt_name)
    else:
        out_dt = mybir.dt.bfloat16 if bf16_out else f32
    out = nc.declare_dram_parameter("out", [b_loc, D0], out_dt, isOutput=True).ap()

    n_tiles = b_loc // TILE
    # row = tile*512 + p*4 + s  (4 rows per partition -> one 1.6MB DMA per tile)
    x_r = x.rearrange("(n p s) f -> n p (s f)", p=SUB, s=NSUB)
    out_r = out.rearrange("(n p s) f -> n p (s f)", p=SUB, s=NSUB)

    with tile.TileContext(nc, num_cores=n_devices, pool_alloc_mode=alloc_mode) as tc:
        with (
            tc.tile_pool(name="consts", bufs=1) as consts,
            tc.tile_pool(name="xin", bufs=xin_bufs) as xin,
            tc.tile_pool(name="xtp", bufs=xtp_bufs) as xtp,
            tc.tile_pool(name="acts", bufs=acts_bufs) as acts,
            tc.tile_pool(name="outp", bufs=outp_bufs) as outp,
            tc.tile_pool(name="psT", bufs=3 if ps_rebal else 2,
                         space="PSUM") as psT,
            tc.tile_pool(name="psMM", bufs=3 if ps_rebal else 2,
                         space="PSUM") as psMM,
            tc.tile_pool(name="psO", bufs=1 if ps_rebal else 2,
                         space="PSUM") as psO,
        ):
            # --- constants ---
            w1t_sb = consts.tile([KCH, NKC, D1], mm_dt)
            nc.sync.dma_start(out=w1t_sb, in_=w1t.rearrange("(c p) m -> p c m", p=KCH))
            w2t_sb = consts.tile([D1, D2], mm_dt)
            nc.sync.dma_start(out=w2t_sb, in_=w2t)
            m3t_sb = consts.tile([D2, D2], mm_dt)
            nc.sync.dma_start(out=m3t_sb, in_=m3t)
            d2t_sb = consts.tile([D2, D1], mm_dt)
            nc.sync.dma_start(out=d2t_sb, in_=d2t)
            d1t_sb = consts.tile([D1, D0], last_dt)
            nc.sync.dma_start(out=d1t_sb, in_=d1t)
            id_sb = consts.tile([SUB, SUB], f32)
            nc.sync.dma_start(out=id_sb, in_=ident)
            id_r = id_sb.bitcast(mybir.dt.float32r)

            do_in = mode in ("full", "dma", "dma_in", "dma_in_cast")
            do_out = mode in ("full", "dma", "dma_out")
            do_compute = mode in ("full", "compute", "xpose", "mm_act")
            o_src = x_c = xt_c = None
            x_rb = x.rearrange("(n p s) f -> n p (s f)", p=SUB,
                               s=NSUB * dma_tiles)
            out_rb = out.rearrange("(n p s) f -> n p (s f)", p=SUB,
                                   s=NSUB * dma_tiles)
            if do_out and not do_compute:
                o_src = consts.tile([SUB, NSUB * dma_tiles * D0], out_dt)
                nc.vector.memset(o_src, 0.25)
            if do_compute and not do_in and mode != "mm_act":
                x_c = consts.tile([SUB, NSUB, D0], f32)
                nc.vector.memset(x_c, 0.125)
            if mode == "mm_act":
                xt_c = [consts.tile([KCH, TILE], mm_dt, name=f"xt_c{i}")
                        for i in range(NKC)]
                for xtc in xt_c:
                    nc.vector.memset(xtc, 0.1)

            rep_ctx = (tc.For_i(0, repeat, 1, staggered_reset=staggered)
                       if repeat > 1 else contextlib.nullcontext())
            with rep_ctx:
              for t in range(n_tiles):
                if not do_compute:
                    if t % dma_tiles:
                        continue
                    tb = t // dma_tiles
                    if mode == "dma_in_cast":
                        x_sb = xin.tile([SUB, NSUB * dma_tiles, D0],
                                        mybir.dt.float16, tag="x", name="x_sb16")
                        nc.gpsimd.dma_start(out=x_sb, in_=x_rb[tb])
                    elif do_in:
                        x_sb = xin.tile([SUB, NSUB * dma_tiles, D0], f32,
                                        tag="x", name="x_sbb")
                        getattr(nc, in_dma_eng if in_dma_eng != "alt" else "sync"
                                ).dma_start(out=x_sb, in_=x_rb[tb])
                    if do_out:
                        getattr(nc, out_dma_eng).dma_start(out=out_rb[tb],
                                                           in_=o_src)
                    continue
                # --- load 512 rows in one DMA: [128, 4, 784] ---
                if do_in:
                    x_sb = xin.tile([SUB, NSUB, D0], f32, tag="x", name="x_sb")
                else:
                    x_sb = x_c
                if not do_in:
                    pass
                elif hi_in:
                    with tc.high_priority():
                        nc.sync.dma_start(out=x_sb, in_=x_r[t])
                elif split_in:
                    xr3 = x_r[t].rearrange("p (s f) -> p s f", s=NSUB)
                    nc.sync.dma_start(out=x_sb[:, 0:2, :], in_=xr3[:, 0:2, :])
                    nc.sync.dma_start(out=x_sb[:, 2:4, :], in_=xr3[:, 2:4, :])
                elif in_dma_eng == "alt":
                    (nc.sync if t % 2 == 0 else nc.scalar).dma_start(
                        out=x_sb, in_=x_r[t])
                else:
                    getattr(nc, in_dma_eng).dma_start(out=x_sb, in_=x_r[t])

                # --- PE-transpose to feature-major: 7 chunks of [112, 512] ---
                xt_sb = []
                for c in range(NKC) if mode != "mm_act" else []:
                    tp = psT.tile([KCH, TILE], f32, tag="psT")
                    for s in range(NSUB):
                        if r_xpose:
                            nc.tensor.transpose(
                                out=tp[:, s * SUB:(s + 1) * SUB]
                                    .bitcast(mybir.dt.float32r),
                                in_=x_sb[:, s, c * KCH:(c + 1) * KCH]
                                    .bitcast(mybir.dt.float32r),
                                identity=id_r,
                            )
                        else:
                            nc.tensor.transpose(
                                out=tp[:, s * SUB:(s + 1) * SUB],
                                in_=x_sb[:, s, c * KCH:(c + 1) * KCH],
                                identity=id_sb,
                            )
                    xt = xtp.tile([KCH, TILE], mm_dt, tag="xt")
                    on_act = (c in xt_on_act if isinstance(xt_on_act, (tuple, list))
                              else xt_on_act)
                    if on_act:
                        nc.scalar.activation(out=xt, in_=tp, func=COPY)
                    else:
                        nc.vector.tensor_copy(xt, tp)
                    xt_sb.append(xt)
                if mode == "xpose":
                    continue
                if mode == "mm_act":
                    xt_sb = xt_c

                # --- L1: h1 = lrelu(W1 @ xT)  [128, 512] ---
                h1_ps = psMM.tile([D1, TILE], f32, tag="mm")
                for c in range(NKC):
                    nc.tensor.matmul(h1_ps, lhsT=w1t_sb[:, c, :], rhs=xt_sb[c],
                                     start=(c == 0), stop=(c == NKC - 1))
                h1_sb = acts.tile([D1, TILE], mm_dt, tag="h1")
                nc.scalar.activation(out=h1_sb, in_=h1_ps, func=LRELU, alpha=0.01)

                # --- L2: h2 = lrelu(W2 @ h1)  [64, 512] ---
                h2_ps = psMM.tile([D2, TILE], f32, tag="mm")
                nc.tensor.matmul(h2_ps, lhsT=w2t_sb, rhs=h1_sb,
                                 start=True, stop=True)
                h2_sb = acts.tile([D2, TILE], mm_dt, tag="h2")
                nc.scalar.activation(out=h2_sb, in_=h2_ps, func=LRELU, alpha=0.01)

                # --- L3 folded: g3 = lrelu((d3 @ W3) @ h2)  [64, 512] ---
                g3_ps = psMM.tile([D2, TILE], f32, tag="mm")
                nc.tensor.matmul(g3_ps, lhsT=m3t_sb, rhs=h2_sb,
                                 start=True, stop=True)
                g3_sb = acts.tile([D2, TILE], mm_dt, tag="g3")
                nc.scalar.activation(out=g3_sb, in_=g3_ps, func=LRELU, alpha=0.01)

                # --- L4: g2 = lrelu(d2 @ g3)  [128, 512] ---
                g2_ps = psMM.tile([D1, TILE], f32, tag="mm")
                nc.tensor.matmul(g2_ps, lhsT=d2t_sb, rhs=g3_sb,
                                 start=True, stop=True)
                g2_sb = acts.tile([D1, TILE], last_dt, tag="g2")
                nc.scalar.activation(out=g2_sb, in_=g2_ps, func=LRELU, alpha=0.01)

                # --- L5: out = g2.T @ d1.T, batch-major via stationary swap.
                # Two matmuls into one 2-bank PSUM tile ([:, :392] in bank 0,
                # [:, 512:904] in bank 1), one strided ACT copy out. ---
                o_sb = outp.tile([SUB, NSUB, D0], out_dt, tag="o")
                if no_l5:
                    nc.vector.tensor_copy(o_sb, x_sb)
                for s in range(NSUB) if not no_l5 else []:
                    g2c = g2_sb[:, s * SUB:(s + 1) * SUB]
                    po = psO.tile([SUB, 1024], f32, tag="po")
                    nc.tensor.matmul(po[:, :HALF], lhsT=g2c, rhs=d1t_sb[:, :HALF],
                                     start=True, stop=True)
                    nc.tensor.matmul(po[:, 512:512 + HALF], lhsT=g2c,
                                     rhs=d1t_sb[:, HALF:], start=True, stop=True)
                    po_v = po.rearrange("p (b r) -> p b r", b=2)[:, :, :HALF]
                    o_v = o_sb[:, s, :].rearrange("p (b r) -> p b r", b=2)
                    if (split_ocopy and s % 2 == 1) or s in ocopy_dve:
                        nc.vector.tensor_copy(o_v, po_v)
                    else:
                        nc.scalar.activation(out=o_v, in_=po_v, func=COPY)
                if do_out:
                    getattr(nc, out_dma_eng).dma_start(out=out_r[t], in_=o_sb)

    nc.finalize()
    return nc


def _build_v3(b_loc=B_LOC, repeat=1, n_devices=N_CORES, mm_dt_name="float32r",
              out_dt_name="float16", out_dma_eng="gpsimd", in_bufs=2,
              outp_bufs=2, xtp_bufs=14, acts_bufs=2, ocopy_dve=(),
              xt_act=(5, 6), staggered=False, interleave=True, sup_tiles=4,
              mode="full"):
    """Software-pipelined variant: 1024-row super-tile DMAs (amortize the
    ~2.7us per-DMA fixed cost), input on the sync HWDGE ring, fp16 output on
    the scalar HWDGE ring, and the next tile's PE transposes interleaved
    between the current tile's layer matmuls so the PE stream stays dense
    (avoids HAM down-throttle to 1.2GHz)."""
    import contextlib
    import concourse.tile as tile
    from concourse import bacc, mybir

    mm_dt = getattr(mybir.dt, mm_dt_name)
    f32 = mybir.dt.float32
    out_dt = getattr(mybir.dt, out_dt_name)
    LRELU = mybir.ActivationFunctionType.Lrelu
    COPY = mybir.ActivationFunctionType.Copy

    SUP = sup_tiles * NSUB    # batch rows per partition per super-tile
    n_sup = b_loc // (SUB * SUP)
    n_tiles = sup_tiles * n_sup

    nc = bacc.Bacc(trn_type="TRN2", target_bir_lowering=False, debug=False,
                   num_devices=n_devices)

    x = nc.declare_dram_parameter("x", [b_loc, D0], f32, isOutput=False).ap()
    w1t = nc.declare_dram_parameter("w1t", [D0, D1], mm_dt, isOutput=False).ap()
    w2t = nc.declare_dram_parameter("w2t", [D1, D2], mm_dt, isOutput=False).ap()
    m3t = nc.declare_dram_parameter("m3t", [D2, D2], mm_dt, isOutput=False).ap()
    d2t = nc.declare_dram_parameter("d2t", [D2, D1], mm_dt, isOutput=False).ap()
    d1t = nc.declare_dram_parameter("d1t", [D1, D0], mm_dt, isOutput=False).ap()
    ident = nc.declare_dram_parameter("ident", [SUB, SUB], f32, isOutput=False).ap()
    out = nc.declare_dram_parameter("out", [b_loc, D0], out_dt, isOutput=True).ap()

    # row = k*1024 + p*8 + s; the in-tile batch permutation cancels between
    # input transposes and output writeback (same (p, s) map both sides).
    x_r = x.rearrange("(n p s) f -> n p (s f)", p=SUB, s=SUP)
    out_r = out.rearrange("(n p s) f -> n p (s f)", p=SUB, s=SUP)

    with tile.TileContext(nc, num_cores=n_devices) as tc:
        with (
            tc.tile_pool(name="consts", bufs=1) as consts,
            tc.tile_pool(name="xin", bufs=in_bufs) as xin,
            tc.tile_pool(name="xtp", bufs=xtp_bufs) as xtp,
            tc.tile_pool(name="acts", bufs=acts_bufs) as acts,
            tc.tile_pool(name="outp", bufs=outp_bufs) as outp,
            tc.tile_pool(name="psT", bufs=2, space="PSUM") as psT,
            tc.tile_pool(name="psMM", bufs=2, space="PSUM") as psMM,
            tc.tile_pool(name="psO", bufs=2, space="PSUM") as psO,
        ):
            w1t_sb = consts.tile([KCH, NKC, D1], mm_dt)
            nc.sync.dma_start(out=w1t_sb, in_=w1t.rearrange("(c p) m -> p c m", p=KCH))
            w2t_sb = consts.tile([D1, D2], mm_dt)
            nc.sync.dma_start(out=w2t_sb, in_=w2t)
            m3t_sb = consts.tile([D2, D2], mm_dt)
            nc.sync.dma_start(out=m3t_sb, in_=m3t)
            d2t_sb = consts.tile([D2, D1], mm_dt)
            nc.sync.dma_start(out=d2t_sb, in_=d2t)
            d1t_sb = consts.tile([D1, D0], mm_dt)
            nc.sync.dma_start(out=d1t_sb, in_=d1t)
            id_sb = consts.tile([SUB, SUB], f32)
            nc.sync.dma_start(out=id_sb, in_=ident)

            rep_ctx = (tc.For_i(0, repeat, 1, staggered_reset=staggered)
                       if repeat > 1 else contextlib.nullcontext())
            with rep_ctx:
                x_sb = {}      # super index -> SBUF tile
                xt_of = {}     # tile index -> list of 7 xt chunks

                def in_dma(k):
                    xk = xin.tile([SUB, SUP, D0], f32, tag="x")
                    nc.sync.dma_start(out=xk, in_=x_r[k])
                    x_sb[k] = xk

                def xp_chunk(g, c):
                    """Transpose chunk c of compute tile g and copy to SBUF."""
                    base = NSUB * (g % sup_tiles)
                    xg = x_sb[g // sup_tiles]
                    tp = psT.tile([KCH, TILE], f32, tag="psT")
                    for s in range(NSUB):
                        nc.tensor.transpose(
                            out=tp[:, s * SUB:(s + 1) * SUB],
                            in_=xg[:, base + s, c * KCH:(c + 1) * KCH],
                            identity=id_sb,
                        )
                    xt = xtp.tile([KCH, TILE], mm_dt, tag="xt")
                    if c in xt_act:
                        nc.scalar.activation(out=xt, in_=tp, func=COPY)
                    else:
                        nc.vector.tensor_copy(xt, tp)
                    xt_of.setdefault(g, []).append(xt)

                in_dma(0)
                for c in range(NKC):
                    xp_chunk(0, c)

                for g in range(n_tiles):
                    nxt = g + 1 if g + 1 < n_tiles else None
                    if g % sup_tiles == 0:
                        o_sb = outp.tile([SUB, sup_tiles, NSUB, D0], out_dt,
                                         tag="o")
                        if g // sup_tiles + 1 < n_sup:
                            in_dma(g // sup_tiles + 1)

                    xt_cur = xt_of.pop(g)

                    # --- L1: h1 = lrelu(W1 @ xT)  [128, 512] ---
                    h1_ps = psMM.tile([D1, TILE], f32, tag="mm")
                    for c in range(NKC):
                        nc.tensor.matmul(h1_ps, lhsT=w1t_sb[:, c, :],
                                         rhs=xt_cur[c],
                                         start=(c == 0), stop=(c == NKC - 1))
                    h1_sb = acts.tile([D1, TILE], mm_dt, tag="h1")
                    nc.scalar.activation(out=h1_sb, in_=h1_ps, func=LRELU,
                                         alpha=0.01)
                    if interleave and nxt is not None:
                        xp_chunk(nxt, 0)
                        xp_chunk(nxt, 1)

                    # --- L2 ---
                    h2_ps = psMM.tile([D2, TILE], f32, tag="mm")
                    nc.tensor.matmul(h2_ps, lhsT=w2t_sb, rhs=h1_sb,
                                     start=True, stop=True)
                    h2_sb = acts.tile([D2, TILE], mm_dt, tag="h2")
                    nc.scalar.activation(out=h2_sb, in_=h2_ps, func=LRELU,
                                         alpha=0.01)
                    if interleave and nxt is not None:
                        xp_chunk(nxt, 2)

                    # --- L3 folded ---
                    g3_ps = psMM.tile([D2, TILE], f32, tag="mm")
                    nc.tensor.matmul(g3_ps, lhsT=m3t_sb, rhs=h2_sb,
                                     start=True, stop=True)
                    g3_sb = acts.tile([D2, TILE], mm_dt, tag="g3")
                    nc.scalar.activation(out=g3_sb, in_=g3_ps, func=LRELU,
                                         alpha=0.01)
                    if interleave and nxt is not None:
                        xp_chunk(nxt, 3)

                    # --- L4 ---
                    g2_ps = psMM.tile([D1, TILE], f32, tag="mm")
                    nc.tensor.matmul(g2_ps, lhsT=d2t_sb, rhs=g3_sb,
                                     start=True, stop=True)
                    g2_sb = acts.tile([D1, TILE], mm_dt, tag="g2")
                    nc.scalar.activation(out=g2_sb, in_=g2_ps, func=LRELU,
                                         alpha=0.01)
                    if interleave and nxt is not None:
                        xp_chunk(nxt, 4)

                    # --- L5: batch-major via stationary swap ---
                    for s in range(NSUB):
                        g2c = g2_sb[:, s * SUB:(s + 1) * SUB]
                        po = psO.tile([SUB, 1024], f32, tag="po")
                        nc.tensor.matmul(po[:, :HALF], lhsT=g2c,
                                         rhs=d1t_sb[:, :HALF],
                                         start=True, stop=True)
                        nc.tensor.matmul(po[:, 512:512 + HALF], lhsT=g2c,
                                         rhs=d1t_sb[:, HALF:],
                                         start=True, stop=True)
                        po_v = po.rearrange("p (b r) -> p b r", b=2)[:, :, :HALF]
                        o_v = o_sb[:, g % sup_tiles, s, :].rearrange(
                            "p (b r) -> p b r", b=2)
                        if s in ocopy_dve:
                            nc.vector.tensor_copy(o_v, po_v)
                        else:
                            nc.scalar.activation(out=o_v, in_=po_v, func=COPY)
                        if interleave and nxt is not None and s < 2:
                            xp_chunk(nxt, 5 + s)
                    if not interleave and nxt is not None:
                        for c in range(NKC):
                            xp_chunk(nxt, c)

                    if g % sup_tiles == sup_tiles - 1:
                        getattr(nc, out_dma_eng).dma_start(
                            out=out_r[g // sup_tiles], in_=o_sb)

    nc.finalize()
    return nc


def _build_v4(b_loc=B_LOC, repeat=1, n_devices=N_CORES, out_dt_name="float16",
              in_dma_eng="gpsimd", out_dma_eng="sync", xin_bufs=4, outp_bufs=4,
              xtp_bufs=14, acts_bufs=2, ocopy_dve=(3,), xt_act=(),
              staggered=False):
    """fp16 datapath: the input DMA casts f32->f16 in flight (SWDGE), so PE
    transposes run at 1 cycle/row (vs 2 for f32), the transpose PSUM and all
    matmul operands are fp16, and SBUF traffic halves.  Input DMA moves to
    the gpsimd queue; output (fp16) to the otherwise-idle sync queue."""
    import contextlib
    import concourse.tile as tile
    from concourse import bacc, mybir

    f32 = mybir.dt.float32
    f16 = mybir.dt.float16
    out_dt = getattr(mybir.dt, out_dt_name)
    LRELU = mybir.ActivationFunctionType.Lrelu
    COPY = mybir.ActivationFunctionType.Copy

    nc = bacc.Bacc(trn_type="TRN2", target_bir_lowering=False, debug=False,
                   num_devices=n_devices)

    x = nc.declare_dram_parameter("x", [b_loc, D0], f32, isOutput=False).ap()
    w1t = nc.declare_dram_parameter("w1t", [D0, D1], f16, isOutput=False).ap()
    w2t = nc.declare_dram_parameter("w2t", [D1, D2], f16, isOutput=False).ap()
    m3t = nc.declare_dram_parameter("m3t", [D2, D2], f16, isOutput=False).ap()
    d2t = nc.declare_dram_parameter("d2t", [D2, D1], f16, isOutput=False).ap()
    d1t = nc.declare_dram_parameter("d1t", [D1, D0], f16, isOutput=False).ap()
    ident = nc.declare_dram_parameter("ident", [SUB, SUB], f16, isOutput=False).ap()
    out = nc.declare_dram_parameter("out", [b_loc, D0], out_dt, isOutput=True).ap()

    n_tiles = b_loc // TILE
    x_r = x.rearrange("(n p s) f -> n p (s f)", p=SUB, s=NSUB)
    out_r = out.rearrange("(n p s) f -> n p (s f)", p=SUB, s=NSUB)

    with tile.TileContext(nc, num_cores=n_devices) as tc:
        with (
            tc.tile_pool(name="consts", bufs=1) as consts,
            tc.tile_pool(name="xin", bufs=xin_bufs) as xin,
            tc.tile_pool(name="xtp", bufs=xtp_bufs) as xtp,
            tc.tile_pool(name="acts", bufs=acts_bufs) as acts,
            tc.tile_pool(name="outp", bufs=outp_bufs) as outp,
            tc.tile_pool(name="psT", bufs=2, space="PSUM") as psT,
            tc.tile_pool(name="psMM", bufs=2, space="PSUM") as psMM,
            tc.tile_pool(name="psO", bufs=2, space="PSUM") as psO,
        ):
            w1t_sb = consts.tile([KCH, NKC, D1], f16)
            nc.sync.dma_start(out=w1t_sb, in_=w1t.rearrange("(c p) m -> p c m", p=KCH))
            w2t_sb = consts.tile([D1, D2], f16)
            nc.sync.dma_start(out=w2t_sb, in_=w2t)
            m3t_sb = consts.tile([D2, D2], f16)
            nc.sync.dma_start(out=m3t_sb, in_=m3t)
            d2t_sb = consts.tile([D2, D1], f16)
            nc.sync.dma_start(out=d2t_sb, in_=d2t)
            d1t_sb = consts.tile([D1, D0], f16)
            nc.sync.dma_start(out=d1t_sb, in_=d1t)
            id_sb = consts.tile([SUB, SUB], f16)
            nc.sync.dma_start(out=id_sb, in_=ident)

            rep_ctx = (tc.For_i(0, repeat, 1, staggered_reset=staggered)
                       if repeat > 1 else contextlib.nullcontext())
            with rep_ctx:
              for t in range(n_tiles):
                # --- load 512 rows, casting f32->f16 in the DMA (SWDGE) ---
                x_sb = xin.tile([SUB, NSUB, D0], f16, tag="x")
                getattr(nc, in_dma_eng).dma_start(out=x_sb, in_=x_r[t])

                # --- PE-transpose (fp16, 1 cyc/row): 7 chunks of [112, 512] ---
                xt_sb = []
                for c in range(NKC):
                    tp = psT.tile([KCH, TILE], f16, tag="psT")
                    for s in range(NSUB):
                        nc.tensor.transpose(
                            out=tp[:, s * SUB:(s + 1) * SUB],
                            in_=x_sb[:, s, c * KCH:(c + 1) * KCH],
                            identity=id_sb,
                        )
                    xt = xtp.tile([KCH, TILE], f16, tag="xt")
                    if c in xt_act:
                        nc.scalar.activation(out=xt, in_=tp, func=COPY)
                    else:
                        nc.vector.tensor_copy(xt, tp)
                    xt_sb.append(xt)

                # --- L1: h1 = lrelu(W1 @ xT)  [128, 512] ---
                h1_ps = psMM.tile([D1, TILE], f32, tag="mm")
                for c in range(NKC):
                    nc.tensor.matmul(h1_ps, lhsT=w1t_sb[:, c, :], rhs=xt_sb[c],
                                     start=(c == 0), stop=(c == NKC - 1))
                h1_sb = acts.tile([D1, TILE], f16, tag="h1")
                nc.scalar.activation(out=h1_sb, in_=h1_ps, func=LRELU, alpha=0.01)

                # --- L2 ---
                h2_ps = psMM.tile([D2, TILE], f32, tag="mm")
                nc.tensor.matmul(h2_ps, lhsT=w2t_sb, rhs=h1_sb,
                                 start=True, stop=True)
                h2_sb = acts.tile([D2, TILE], f16, tag="h2")
                nc.scalar.activation(out=h2_sb, in_=h2_ps, func=LRELU, alpha=0.01)

                # --- L3 folded ---
                g3_ps = psMM.tile([D2, TILE], f32, tag="mm")
                nc.tensor.matmul(g3_ps, lhsT=m3t_sb, rhs=h2_sb,
                                 start=True, stop=True)
                g3_sb = acts.tile([D2, TILE], f16, tag="g3")
                nc.scalar.activation(out=g3_sb, in_=g3_ps, func=LRELU, alpha=0.01)

                # --- L4 ---
                g2_ps = psMM.tile([D1, TILE], f32, tag="mm")
                nc.tensor.matmul(g2_ps, lhsT=d2t_sb, rhs=g3_sb,
                                 start=True, stop=True)
                g2_sb = acts.tile([D1, TILE], f16, tag="g2")
                nc.scalar.activation(out=g2_sb, in_=g2_ps, func=LRELU, alpha=0.01)

                # --- L5: batch-major via stationary swap ---
                o_sb = outp.tile([SUB, NSUB, D0], out_dt, tag="o")
                for s in range(NSUB):
                    g2c = g2_sb[:, s * SUB:(s + 1) * SUB]
                    po = psO.tile([SUB, 1024], f32, tag="po")
                    nc.tensor.matmul(po[:, :HALF], lhsT=g2c, rhs=d1t_sb[:, :HALF],
                                     start=True, stop=True)
                    nc.tensor.matmul(po[:, 512:512 + HALF], lhsT=g2c,
                                     rhs=d1t_sb[:, HALF:], start=True, stop=True)
                    po_v = po.rearrange("p (b r) -> p b r", b=2)[:, :, :HALF]
                    o_v = o_sb[:, s, :].rearrange("p (b r) -> p b r", b=2)
                    if s in ocopy_dve:
                        nc.vector.tensor_copy(o_v, po_v)
                    else:
                        nc.scalar.activation(out=o_v, in_=po_v, func=COPY)
                getattr(nc, out_dma_eng).dma_start(out=out_r[t], in_=o_sb)

    nc.finalize()
    return nc


def _host_weights(W1, W2, W3, np_dtype=np.float32):
    def pinv(W):
        u, s, vh = np.linalg.svd(W.astype(np.float64), full_matrices=False)
        return (vh.T * (1.0 / s)) @ u.T

    d1, d2, d3 = pinv(W1), pinv(W2), pinv(W3)
    f = np_dtype
    return {
        "w1t": np.ascontiguousarray(W1.T, dtype=f),
        "w2t": np.ascontiguousarray(W2.T, dtype=f),
        "m3t": np.ascontiguousarray((d3 @ W3.astype(np.float64)).T, dtype=f),
        "d2t": np.ascontiguousarray(d2.T, dtype=f),
        "d1t": np.ascontiguousarray(d1.T, dtype=f),
        "ident": np.eye(SUB, dtype=f),
    }


def host_weight_dtype(cfg=None):
    cfg = CONFIG if cfg is None else cfg
    return np.float16 if cfg.get("arch") == "v4" else np.float32


_NC_CACHE = {}

# Tuned configuration: fp16 output (tolerance is 2e-2; fp16 adds ~3e-4),
# output DMA on the otherwise-idle GPSIMD queue, and copy work rebalanced
# off the DVE (whose ops pay a pipeline-DRAIN ≈ op duration): DVE keeps 5
# transpose-chunk copies; ACT takes 2 chunk copies + all 4 output copies.
CONFIG = dict(
    arch="v2",
    out_dt_name="float16",
    out_dma_eng="gpsimd",
    ocopy_dve=(),
    xt_on_act=(5, 6),
)


def _get_nc(key=None):
    if key not in _NC_CACHE:
        _NC_CACHE[key] = _build_nc(B_LOC, **CONFIG)
    return _NC_CACHE[key]


def kernel(x, W1, W2, W3):
    from concourse.bass_utils import run_bass_kernel_spmd

    x = np.ascontiguousarray(x, dtype=np.float32)
    w = _host_weights(np.asarray(W1), np.asarray(W2), np.asarray(W3),
                      host_weight_dtype())
    nc = _get_nc()
    in_maps = [
        {"x": x[i * B_LOC:(i + 1) * B_LOC], **w} for i in range(N_CORES)
    ]
    res = run_bass_kernel_spmd(nc, in_maps, core_ids=list(range(N_CORES)))
    return np.concatenate(
        [np.asarray(res.results[i]["out"], dtype=np.float32) for i in range(N_CORES)],
        axis=0,
    )

